# revision 3
# baseline (speedup 1.0000x reference)
"""GroupNorm + full spatial self-attention block on 8 Trainium2 NeuronCores.

Strategy: data parallelism over batch (B=32 -> 4 images per core, zero
collectives). All five big matmul groups (q/k/v projections, scores,
attention-apply, output projection) run in fp8-e4m3 with
perf_mode=DoubleRow: each matmul contracts K=256 (two 128-row tiles,
2 fp8 weights per PE cell) in the same ~216ns an N=512 bf16 matmul
takes -> 2x the bf16 matmul roofline.

Numerics: scores have heavy tails (max ~15), so exp stays in bf16
(softmax-without-max, no overflow). The softmax weights are then
normalized on the vector engine: E_n = e4m3(32 * E / denom), with the
32x keeping near-uniform weights (~1/1024) above the fp8 subnormal
flush. Denominators come from a bf16 ones-matmul whose ones are 1/32,
so the reciprocal directly yields 32/denom. The attention output is
rescaled (1/8) into fp8 for the output projection (Wn pre-scaled 2048x
on the host); the epilogue multiplies by 1/8192 and adds the residual
in one fused scalar_tensor_tensor. bneff (Wn^T bv + bn) enters via a
K=1 bf16 matmul row appended to the output projection accumulation.

GroupNorm: per-channel bn_stats/bn_aggr on the SBUF-resident x (no
second HBM read of x), then a tiny fp32 matmul reduces 16-channel
blocks into per-group stats.

Images are software-pipelined: image b+1's x-load/stats/affine are
issued between image b's projection and scores phases so the PE never
waits at image boundaries.
"""

import numpy as np
import ml_dtypes

import concourse.bass as bass
import concourse.tile as tile
from concourse import mybir
from concourse.vector_clock import ScopedClock
import concourse.bass2jax as _bass2jax
import json as _json

F32 = mybir.dt.float32
BF16 = mybir.dt.bfloat16
F8 = mybir.dt.float8e4
AF = mybir.ActivationFunctionType
OP = mybir.AluOpType
DR = mybir.MatmulPerfMode.DoubleRow

B, C, H, W = 32, 512, 32, 32
HW = H * W                      # 1024 spatial positions
NCORES = 8
BL = B // NCORES                # 4 images per core
G = 32                          # groups
GS = C // G                     # 16 channels per group
EPS = 1e-5
P = 128
KC = C // P                     # 4 channel chunks
QT = HW // P                    # 8 key tiles
NH = HW // 512                  # 2 matmul halves of the spatial dim
GL = G // KC                    # 8 groups per channel chunk
SCALE = float(C) ** -0.5
KAPPA = 32.0                    # softmax weight scale (via ones_col = 1/32)
WN_SCALE = 2048.0               # host-side Wn prescale for fp8
HA_SCALE = 0.125                # attention-out prescale into fp8
EPI_SCALE = 1.0 / 8192.0        # 1/(WN_SCALE*KAPPA*HA_SCALE)


# ---------------------------------------------------------------------------
# Workarounds for this walrus build, which encodes at most ONE sync wait per
# instruction. (1) Tile's exit path piles every final sem wait onto a single
# Drain; emit standalone waits instead. (2) Split any remaining multi-wait
# instruction in the BIR into standalone EventSemaphore waits.

def _patched_drain_and_barrier(self, tick_clock, wait_clock):
    nc = self.nc
    probe = nc.sync.nop(nofuse=True)
    wait_clock.add_sem_waits(probe.ins, ScopedClock({None: tick_clock.global_clock}))
    si = probe.ins.sync_info
    waits = list(si.on_wait) if si is not None else []
    if si is not None:
        probe.ins.sync_info = mybir.SyncInfo(on_wait=[], on_update=list(si.on_update))
    name2sem = {s.name: s for s in self.sems.allocated().values()}
    for w in waits:
        nc.sync.wait_ge(name2sem[w.ant_name], w.wait_value)
    nc.sync.drain()
    nc.all_engine_barrier(sem_only=True)
    popped = nc._tile_sem_poison_stack.pop()
    assert popped is self._sem_poison
    nc.clear_and_free_semaphores(list(self.sems.allocated().values()))
    nc.all_engine_barrier(sem_only=True)


tile.TileContext._drain_and_barrier = _patched_drain_and_barrier

_orig_compile_bir_kernel = _bass2jax.compile_bir_kernel


def _split_multiwait_bir(bir_bytes):
    bir = _json.loads(bir_bytes)
    for fn in bir.get("functions", []):
        for blk in fn.get("blocks", []):
            insts = blk.get("instructions")
            if not insts:
                continue
            out = []
            for ins in insts:
                si = ins.get("sync_info")
                waits = (si or {}).get("on_wait") or []
                if len(waits) > 1:
                    for j, w in enumerate(waits[:-1]):
                        out.append({
                            "debug": ins.get("debug"),
                            "engine": ins["engine"],
                            "ins": [],
                            "outs": [],
                            "name": f"{ins['name']}-xw{j}",
                            "opcode": "EventSemaphore",
                            "sync_info": {"on_update": [], "on_wait": [w]},
                        })
                    si["on_wait"] = [waits[-1]]
                out.append(ins)
            blk["instructions"] = out
    return _json.dumps(bir).encode()


def _compile_bir_kernel_splitwaits(ant_bir_str, compile_dir_path, **kwargs):
    return _orig_compile_bir_kernel(
        _split_multiwait_bir(ant_bir_str), compile_dir_path, **kwargs
    )


_bass2jax.compile_bir_kernel = _compile_bir_kernel_splitwaits


# ---------------------------------------------------------------------------

class _Consts:
    pass


def _build_program():
    nc = bass.Bass()
    xs = nc.dram_tensor("xs", [BL, C, HW], F32, kind="ExternalInput")
    wq = nc.dram_tensor("wq", [P, KC, C], F8, kind="ExternalInput")
    wk = nc.dram_tensor("wk", [P, KC, C], F8, kind="ExternalInput")
    wv = nc.dram_tensor("wv", [P, KC, C], F8, kind="ExternalInput")
    wn = nc.dram_tensor("wn", [P, KC, C], F8, kind="ExternalInput")
    bqd = nc.dram_tensor("bq", [C], F32, kind="ExternalInput")
    bkd = nc.dram_tensor("bk", [C], F32, kind="ExternalInput")
    bnrd = nc.dram_tensor("bneffr", [1, C], BF16, kind="ExternalInput")
    blkd = nc.dram_tensor("blkones", [P, GL], F32, kind="ExternalInput")
    out = nc.dram_tensor("out", [BL, C, HW], F32, kind="ExternalOutput")

    with tile.TileContext(nc) as tc:
        with (
            tc.tile_pool(name="const", bufs=1) as constp,
            tc.tile_pool(name="img", bufs=2) as img,
            tc.tile_pool(name="sb3", bufs=3) as sb3,
            tc.tile_pool(name="dram", bufs=2, space="DRAM") as dramp,
            tc.tile_pool(name="psA", bufs=4, space="PSUM") as psA,
            tc.tile_pool(name="psS", bufs=1, space="PSUM") as psS,
            tc.tile_pool(name="psG", bufs=2, space="PSUM") as psG,
        ):
            cs = _Consts()
            # image-0 x first so stats can start immediately
            pre0_x = []
            for q in range(KC):
                xt = img.tile([P, HW], F32, tag=f"xch{q}", name=f"xch{q}")
                nc.sync.dma_start(xt[:], xs[0, P * q:P * (q + 1), :])
                pre0_x.append(xt)

            cs.eps = constp.tile([P, 1], F32, tag="eps", name="eps")
            nc.vector.memset(cs.eps[:], EPS)
            cs.ones_col = constp.tile([P, 1], BF16, tag="onesc", name="onesc")
            nc.vector.memset(cs.ones_col[:], 1.0 / KAPPA)
            cs.ones_row = constp.tile([1, 512], BF16, tag="onesr", name="onesr")
            nc.vector.memset(cs.ones_row[:], 1.0)
            cs.blk = constp.tile([P, GL], F32, tag="blk", name="blk")
            nc.sync.dma_start(cs.blk[:], blkd[:])
            cs.w = {}
            for name, dram in (("wq", wq), ("wk", wk), ("wv", wv), ("wn", wn)):
                t = constp.tile([P, KC, C], F8, tag=name, name=name)
                nc.sync.dma_start(t[:], dram[:])
                cs.w[name] = t
            cs.bq = constp.tile([P, KC], F32, tag="bq", name="bq")
            nc.sync.dma_start(cs.bq[:], bqd[:].rearrange("(kc p) -> p kc", p=P))
            cs.bk = constp.tile([P, KC], F32, tag="bk", name="bk")
            nc.sync.dma_start(cs.bk[:], bkd[:].rearrange("(kc p) -> p kc", p=P))
            cs.bnr = constp.tile([1, C], BF16, tag="bnr", name="bnr")
            nc.sync.dma_start(cs.bnr[:], bnrd[:])

            pre = _pre(nc, 0, xs, cs, img, psG, pre0_x)
            for b in range(BL):
                nxt = _attn_front(nc, b, cs, pre, img, psA, psS, dramp)
                pre_next = _pre(nc, b + 1, xs, cs, img, psG) if b + 1 < BL else None
                _attn_back(nc, b, cs, pre, nxt, out, img, sb3, psA)
                pre = pre_next

    return nc


def _pre(nc, b, xs, cs, img, psG, xch=None):
    """x load + groupnorm stats + affine -> h8 fp8 [P, KC, HW]."""
    if xch is None:
        xch = []
        for q in range(KC):
            xt = img.tile([P, HW], F32, tag=f"xch{q}", name=f"xch{q}")
            nc.sync.dma_start(xt[:], xs[b, P * q:P * (q + 1), :])
            xch.append(xt)

    # per-channel stats per chunk; e2t[:, q, :] = (mean, mean^2+var)
    st6 = img.tile([P, KC, 2, 6], F32, tag="st6", name="st6")
    mv = img.tile([P, KC, 2], F32, tag="mv", name="mv")
    e2t = img.tile([P, KC, 2], F32, tag="e2t", name="e2t")
    for q in range(KC):
        nc.vector.bn_stats(out=st6[:, q, 0, :], in_=xch[q][:, 0:512])
        nc.vector.bn_stats(out=st6[:, q, 1, :], in_=xch[q][:, 512:1024])
        nc.vector.bn_aggr(out=mv[:, q, :], in_=st6[:, q, :, :])
        nc.vector.tensor_copy(out=e2t[:, q, 0:1], in_=mv[:, q, 0:1])
        nc.vector.scalar_tensor_tensor(
            e2t[:, q, 1:2], mv[:, q, 0:1], mv[:, q, 0:1], mv[:, q, 1:2],
            OP.mult, OP.add)

    # reduce 16-channel blocks -> per-group sums [GL, (q, stat)]
    gps = psG.tile([GL, KC * 2], F32, tag="gps", name="gps")
    nc.tensor.matmul(gps[:], cs.blk[:], e2t[:], start=True, stop=True)
    gw = img.tile([GL, KC, 2], F32, tag="gw", name="gw")
    nc.vector.tensor_scalar_mul(gw[:], gps[:], 1.0 / GS)
    musq = img.tile([GL, KC], F32, tag="musq", name="musq")
    nc.vector.tensor_tensor(musq[:], gw[:, :, 0], gw[:, :, 0], OP.mult)
    var = img.tile([GL, KC], F32, tag="var", name="var")
    nc.vector.tensor_tensor(var[:], gw[:, :, 1], musq[:], OP.subtract)
    std = img.tile([GL, KC], F32, tag="std", name="std")
    nc.scalar.activation(out=std[:], in_=var[:], func=AF.Sqrt,
                         bias=cs.eps[:GL])
    rs = img.tile([GL, KC], F32, tag="rs", name="rs")
    nc.vector.reciprocal(out=rs[:], in_=std[:])
    sh = img.tile([GL, KC], F32, tag="sh", name="sh")
    nc.vector.scalar_tensor_tensor(sh[:], gw[:, :, 0], -1.0, rs[:],
                                   OP.mult, OP.mult)
    gsb = img.tile([GL, KC, GS, 2], F32, tag="gsb", name="gsb")
    nc.vector.tensor_copy(out=gsb[:, :, :, 0],
                          in_=rs[:, :, None].to_broadcast((GL, KC, GS)))
    nc.vector.tensor_copy(out=gsb[:, :, :, 1],
                          in_=sh[:, :, None].to_broadcast((GL, KC, GS)))

    # h8 = fp8(x * scale + shift), per chunk on gpsimd
    h8 = img.tile([P, KC, HW], F8, tag="h8", name="h8")
    st = []
    for q in range(KC):
        s = img.tile([P, 2], F32, tag=f"st{q}", name=f"st{q}")
        nc.sync.dma_start(s[:], gsb[:, q, :, :])
        st.append(s)
    for q in range(KC):
        nc.gpsimd.tensor_scalar(out=h8[:, q, :], in0=xch[q][:],
                                scalar1=st[q][:, 0:1], scalar2=st[q][:, 1:2],
                                op0=OP.mult, op1=OP.add)
    return xch, h8


def _attn_front(nc, b, cs, pre, img, psA, psS, dramp):
    """q/k proj, scores+exp, rowsums, R chain, v proj."""
    xch, h8 = pre

    q8 = img.tile([P, KC, HW], F8, tag="q8", name="q8")
    k8 = img.tile([P, KC, HW], F8, tag="k8", name="k8")
    for wname, dst, bias, eng in (("wq", q8, cs.bq, "v"), ("wk", k8, cs.bk, "s")):
        w = cs.w[wname]
        for m in range(KC):
            for hh in range(NH):
                ps = psA.tile([P, 512], F32, tag="mm", name="mm")
                for kp in range(2):
                    nc.tensor.matmul(
                        ps[:], w[:, 2 * kp:2 * kp + 2, P * m:P * (m + 1)],
                        h8[:, 2 * kp:2 * kp + 2, 512 * hh:512 * (hh + 1)],
                        start=(kp == 0), stop=(kp == 1), perf_mode=DR)
                o = dst[:, m, 512 * hh:512 * (hh + 1)]
                if eng == "v":
                    nc.vector.tensor_scalar_add(o, ps[:], bias[:, m:m + 1])
                else:
                    nc.scalar.activation(out=o, in_=ps[:], func=AF.Identity,
                                         bias=bias[:, m:m + 1])

    # scores (transposed: S_T[key q, query p]) -> exp bf16
    E = img.tile([P, QT, HW], BF16, tag="E", name="E")
    for i in range(QT):
        for hh in range(NH):
            ps = psA.tile([P, 512], F32, tag="mm", name="mm")
            for kp in range(2):
                nc.tensor.matmul(
                    ps[:], k8[:, 2 * kp:2 * kp + 2, P * i:P * (i + 1)],
                    q8[:, 2 * kp:2 * kp + 2, 512 * hh:512 * (hh + 1)],
                    start=(kp == 0), stop=(kp == 1), perf_mode=DR)
            nc.scalar.activation(out=E[:, i, 512 * hh:512 * (hh + 1)],
                                 in_=ps[:], func=AF.Exp, scale=SCALE)

    # denominators/32 via ones(=1/32) matmul, lagged behind the exps
    sums_ps = [psS.tile([1, 512], F32, tag=f"sums{hh}", name=f"sums{hh}")
               for hh in range(NH)]
    for i in range(QT):
        for hh in range(NH):
            nc.tensor.matmul(sums_ps[hh][:], cs.ones_col[:],
                             E[:, i, 512 * hh:512 * (hh + 1)],
                             start=(i == 0), stop=(i == QT - 1))

    # R = 32/denom, broadcast to all partitions via DRAM roundtrip (bf16)
    srow = img.tile([1, HW], F32, tag="srow", name="srow")
    for hh in range(NH):
        nc.scalar.copy(out=srow[:, 512 * hh:512 * (hh + 1)],
                       in_=sums_ps[hh][:])
    rscrA = dramp.tile([1, HW], F32, tag="rscrA", name="rscrA")
    nc.sync.dma_start(rscrA[:], srow[:])
    sblk = img.tile([P, QT], F32, tag="sblk", name="sblk")
    nc.sync.dma_start(sblk[:], rscrA[:])
    rblk = img.tile([P, QT], BF16, tag="rblk", name="rblk")
    with nc.allow_low_precision(reason="R in bf16: 0.4% on softmax scale is fine"):
        nc.vector.reciprocal(out=rblk[:], in_=sblk[:])
    rscrB = dramp.tile([1, HW], BF16, tag="rscrB", name="rscrB")
    nc.sync.dma_start(rscrB[:], rblk[:])
    R_sb = img.tile([P, HW], BF16, tag="Rsb", name="Rsb")
    nc.sync.dma_start(R_sb[:], rscrB[:].partition_broadcast(P))

    # v projection (fills the PE while the R roundtrip completes)
    v8 = img.tile([P, QT, 512], F8, tag="v8", name="v8")
    for i in range(QT):
        ps = psA.tile([P, 512], F32, tag="mm", name="mm")
        for kp in range(2):
            nc.tensor.matmul(ps[:], h8[:, 2 * kp:2 * kp + 2, P * i:P * (i + 1)],
                             cs.w["wv"][:, 2 * kp:2 * kp + 2, :],
                             start=(kp == 0), stop=(kp == 1), perf_mode=DR)
        nc.scalar.copy(out=v8[:, i, :], in_=ps[:])

    return E, R_sb, v8


def _attn_back(nc, b, cs, pre, front, out, img, sb3, psA):
    """E_n, apply, hA8, outproj, epilogue, output DMA."""
    xch, h8 = pre
    E, R_sb, v8 = front

    En8 = img.tile([P, QT, HW], F8, tag="En8", name="En8")
    for i in range(QT):
        nc.vector.tensor_tensor(En8[:, i, :], E[:, i, :], R_sb[:], OP.mult)

    hA8 = img.tile([P, KC, HW], F8, tag="hA8", name="hA8")
    for m in range(KC):
        for hh in range(NH):
            ps = psA.tile([P, 512], F32, tag="mm", name="mm")
            for tp in range(QT // 2):
                nc.tensor.matmul(
                    ps[:], v8[:, 2 * tp:2 * tp + 2, P * m:P * (m + 1)],
                    En8[:, 2 * tp:2 * tp + 2, 512 * hh:512 * (hh + 1)],
                    start=(tp == 0), stop=(tp == QT // 2 - 1), perf_mode=DR)
            nc.scalar.mul(out=hA8[:, m, 512 * hh:512 * (hh + 1)], in_=ps[:],
                          mul=HA_SCALE)

    for m in range(KC):
        for hh in range(NH):
            ps = psA.tile([P, 512], F32, tag="mm", name="mm")
            for kp in range(2):
                nc.tensor.matmul(
                    ps[:], cs.w["wn"][:, 2 * kp:2 * kp + 2, P * m:P * (m + 1)],
                    hA8[:, 2 * kp:2 * kp + 2, 512 * hh:512 * (hh + 1)],
                    start=(kp == 0), stop=False, perf_mode=DR)
            nc.tensor.matmul(ps[:], cs.bnr[:, P * m:P * (m + 1)],
                             cs.ones_row[:], start=False, stop=True)
            osb = sb3.tile([P, 512], F32, tag="osb", name="osb")
            nc.vector.scalar_tensor_tensor(
                osb[:], ps[:], EPI_SCALE,
                xch[m][:, 512 * hh:512 * (hh + 1)], OP.mult, OP.add)
            nc.sync.dma_start(
                out[b, P * m:P * (m + 1), 512 * hh:512 * (hh + 1)], osb[:])


_cached_nc = None


def _get_program():
    global _cached_nc
    if _cached_nc is None:
        _cached_nc = _build_program()
    return _cached_nc


def _run(inputs, trace=False, trace_cores=None):
    """Shard, run on 8 cores, gather. Returns (out [B,C,H,W] f32, exec_ns)."""
    from concourse.bass_utils import run_bass_kernel_spmd

    x = np.asarray(inputs["x"], dtype=np.float32).reshape(B, C, HW)
    f8 = ml_dtypes.float8_e4m3fn
    bf = ml_dtypes.bfloat16

    def shuf(w, scale=1.0):
        # [C, C] -> [P, KC, C]: each partition's weight bytes contiguous
        w = np.clip(np.asarray(w, dtype=np.float32) * scale, -240, 240).astype(f8)
        return np.ascontiguousarray(w.reshape(KC, P, C).transpose(1, 0, 2))

    wq8 = shuf(inputs["Wq"])
    wk8 = shuf(inputs["Wk"])
    wv8 = shuf(inputs["Wv"])
    wn8 = shuf(inputs["Wn"], WN_SCALE)
    bq = np.asarray(inputs["bq"], dtype=np.float32)
    bk = np.asarray(inputs["bk"], dtype=np.float32)
    bv = np.asarray(inputs["bv"], dtype=np.float32)
    bn = np.asarray(inputs["bn"], dtype=np.float32)
    wn32 = np.asarray(inputs["Wn"], dtype=np.float32)
    # bneff enters the outproj psum via a K=1 matmul; epilogue divides by 8192
    bneffr = ((wn32.T @ bv + bn) / EPI_SCALE).astype(bf).reshape(1, C)

    blkones = np.zeros((P, GL), dtype=np.float32)
    for p in range(P):
        blkones[p, p // GS] = 1.0

    shared = {"wq": wq8, "wk": wk8, "wv": wv8, "wn": wn8,
              "bq": bq, "bk": bk, "bneffr": bneffr, "blkones": blkones}
    in_maps = []
    for i in range(NCORES):
        m = dict(shared)
        m["xs"] = np.ascontiguousarray(x[BL * i:BL * (i + 1)])
        in_maps.append(m)

    nc = _get_program()
    kwargs = {}
    if trace:
        kwargs["trace"] = True
        if trace_cores is not None:
            kwargs["trace_cores"] = trace_cores
    res = run_bass_kernel_spmd(nc, in_maps, core_ids=list(range(NCORES)),
                               **kwargs)
    outs = [res.results[i]["out"] for i in range(NCORES)]
    full = np.concatenate(outs, axis=0).reshape(B, C, H, W)
    return full.astype(np.float32), res.exec_time_ns


def kernel(**inputs):
    out, _ = _run(inputs, trace=False)
    return out
